# revision 2
# baseline (speedup 1.0000x reference)
"""MoE routing kernel for Trainium2, 8 NeuronCores, expert-parallel.

V3: V2 (3-stream error-compensated fp8 DoubleRow matmuls) plus:

- Owner rebalancing: token "ownership" (which core outputs which token's
  combined y) is a free permutation. A greedy balancer assigns owners so
  every (expert, owner) bucket count approaches ceil(N_e/8), shrinking the
  bucket pad CAP from max_bucket (160 here) to ~144 => SR=8*CAP drops ~10%
  and ALL PE work (both matmuls scale with SR) drops with it. The host
  un-permutes rows on output assembly.
- mm1 consumer rebalance (b1==0 fast path): the two ACT ops and the two
  DVE stt ops run PAIRED over [P, 2, L] (per f-pair, halving per-op init
  overhead); the e4m3 hi split is a gpsimd cast-DMA (SWDGE dtype-cast,
  ~1us Pool-engine descgen instead of ~3.2us of Pool ALU copy); only the
  e5m2 residual sub stays on Pool ALU, software-pipelined one f-pair
  behind the cast so the Pool queue never head-blocks on the DMA sem.
  Per-f-pair engine busy (SR=1152): PE 5760, ACT ~4950, DVE ~3970,
  Pool ~4300 -> PE-bound.
- mm2 stream order (ah@W2h, ah@W2l, al@W2h) so the last-produced operand
  (al of the final f-pair, which trails its deferred sub) is needed last.
- b2==0 fast path drops the 4 per-tg bias adds from every combine tail.
- First x chunk is 128 columns so the first matmul starts ~2us earlier.

Matmul scheme (unchanged from V2): A = Ah + Al with Ah=e4m3(A),
Al=e5m2(A-Ah); A@B ~= Ah@Bh + Al@Bh + Ah@Bl as fp8 DoubleRow (0.5 cyc/col,
256-row contraction) = 1.5 cyc/col per 256 rows vs bf16's 2.0. W1/W2
pre-scaled by 32 for e4m3 normal range; activations computed as
temp = 32*act(h) via Erf/Sigmoid table tricks (function choice is per-core
DATA through scale operands); psum2 descaled by wcol/1024 into the bf16
all-to-all payload; owners gather their two contribution rows per token.
"""

import numpy as np
import ml_dtypes

D_MODEL, D_FF, N_EXPERTS, TOP_K = 1024, 4096, 8, 2
B, S = 2, 2048
T = B * S
NCORES = 8
P = 128
SHARD = T // NCORES     # 512 tokens owned per core
FD = D_FF // P          # 32 f-tiles
FD2 = FD // 2           # 16 f-pairs
KD = D_MODEL // P       # 8 k-tiles (d_model)
KD2 = KD // 2           # 4 k-pairs
TG = SHARD // P         # 4 owned-token tiles
MAX_CAP = 160           # SR = 8*CAP <= 1280 (SBUF residency bound)
WSCALE = 32.0           # fp8 pre-scale for W1/W2

_prog_cache = {}
_wprep_cache = {}

_bf16 = ml_dtypes.bfloat16
_e4 = ml_dtypes.float8_e4m3
_e5 = ml_dtypes.float8_e5m2


def _chunks(n):
    """First chunk small (early PE start), then 512s."""
    out = []
    o = 0
    first = min(128, n)
    out.append((0, first))
    o = first
    while o < n:
        L = min(512, n - o)
        out.append((o, L))
        o += L
    return out


def _build_program(CAP, b1z, b2z):
    import concourse.tile as tile
    from concourse import bacc, mybir, library_config

    f32 = mybir.dt.float32
    bf16 = mybir.dt.bfloat16
    fp8h = mybir.dt.float8e4
    fp8l = mybir.dt.float8e5
    i16 = mybir.dt.int16
    DR = mybir.MatmulPerfMode.DoubleRow
    SR = NCORES * CAP
    G = SR // P
    CH = _chunks(SR)

    nc = bacc.Bacc("TRN2", target_bir_lowering=False, debug=False,
                   num_devices=NCORES)

    xTh = nc.dram_tensor("xTh", [P, KD2, 2, SR], fp8h, kind="ExternalInput").ap()
    xTl = nc.dram_tensor("xTl", [P, KD2, 2, SR], fp8l, kind="ExternalInput").ap()
    W1h = nc.dram_tensor("W1h", [FD // 2, P, 2, KD2 * 2 * P], fp8h,
                         kind="ExternalInput").ap()
    W1l = nc.dram_tensor("W1l", [FD // 2, P, 2, KD2 * 2 * P], fp8l,
                         kind="ExternalInput").ap()
    W2h = nc.dram_tensor("W2h", [FD2 // 2, P, 2, 2, D_MODEL], fp8h,
                         kind="ExternalInput").ap()
    W2l = nc.dram_tensor("W2l", [FD2 // 2, P, 2, 2, D_MODEL], fp8l,
                         kind="ExternalInput").ap()
    acts = nc.dram_tensor("acts", [P, 2], f32, kind="ExternalInput").ap()
    if not b1z:
        b1g = nc.dram_tensor("b1g", [P, FD], f32, kind="ExternalInput").ap()
        b1s = nc.dram_tensor("b1s", [P, FD], f32, kind="ExternalInput").ap()
        b1a = nc.dram_tensor("b1a", [P, FD], f32, kind="ExternalInput").ap()
    if not b2z:
        b2bc = nc.dram_tensor("b2bc", [P, D_MODEL], f32,
                              kind="ExternalInput").ap()
    wct = nc.dram_tensor("wct", [P, G], f32, kind="ExternalInput").ap()
    idx12 = nc.dram_tensor("idx12", [P, 2 * SHARD // 16], i16,
                           kind="ExternalInput").ap()
    y_shard = nc.dram_tensor("y_shard", [SHARD, D_MODEL], bf16,
                             kind="ExternalOutput").ap()

    # collective payload split: a half, then two telescoping quarters
    QWS = (512, 384, 128)
    QO = (0, 512, 896)
    send_q = [nc.dram_tensor(f"send_q{q}", [SR, QWS[q]], bf16).ap()
              for q in range(3)]
    recv_q = [nc.dram_tensor(f"recv_q{q}", [SR, QWS[q]], bf16).ap()
              for q in range(3)]

    with tile.TileContext(nc) as tc:
        with (
            tc.tile_pool(name="xtp", bufs=1) as xtp,
            tc.tile_pool(name="atp", bufs=1) as atp,
            tc.tile_pool(name="w1ph", bufs=3) as w1ph,
            tc.tile_pool(name="w1p", bufs=2) as w1p,
            tc.tile_pool(name="w2p", bufs=1) as w2p,
            tc.tile_pool(name="smalls", bufs=1) as smalls,
            tc.tile_pool(name="actp", bufs=2) as actp,
            tc.tile_pool(name="sndp", bufs=3) as sndp,
            tc.tile_pool(name="tmpp", bufs=2) as tmpp,
            tc.tile_pool(name="cmb", bufs=1) as cmb,
            tc.tile_pool(name="psm1", bufs=2, space="PSUM") as psm1,
            tc.tile_pool(name="psm2", bufs=4, space="PSUM") as psm2,
        ):
            nc.gpsimd.load_library(library_config.mlp)

            xth = xtp.tile([P, KD2, 2, SR], fp8h, tag="xh", name="xh")
            xtl = xtp.tile([P, KD2, 2, SR], fp8l, tag="xl", name="xl")

            ah8, al8 = [], []
            for p2 in range(FD2):
                ah8.append(atp.tile([P, 2, SR], fp8h, tag=f"ah{p2}",
                                    name=f"ah{p2}"))
                al8.append(atp.tile([P, 2, SR], fp8l, tag=f"al{p2}",
                                    name=f"al{p2}"))

            def load_w1h(fp):
                w1fh = w1ph.tile([P, 2, KD2, 2, P], fp8h, tag="w1fh")
                nc.sync.dma_start(out=w1fh[:], in_=W1h[fp].rearrange(
                    "p c (a b q) -> p c a b q", a=KD2, b=2))
                return w1fh

            def load_w1l(fp):
                w1fl = w1p.tile([P, 2, KD2, 2, P], fp8l, tag="w1fl")
                nc.sync.dma_start(out=w1fl[:], in_=W1l[fp].rearrange(
                    "p c (a b q) -> p c a b q", a=KD2, b=2))
                return w1fl

            # FIFO prefetch: hi tiles 2 pairs ahead, lo 1 pair ahead.
            # DMA issue order front-loads what the PE needs first: W1h(0),
            # x chunk0 (128 cols), act scales, then the rest.
            w1h_q = [load_w1h(0)]
            (o0, L0) = CH[0]
            nc.sync.dma_start(out=xth[:, :, :, o0:o0 + L0],
                              in_=xTh[:, :, :, o0:o0 + L0])
            nc.sync.dma_start(out=xtl[:, :, :, o0:o0 + L0],
                              in_=xTl[:, :, :, o0:o0 + L0])
            actt = smalls.tile([P, 2], f32, tag="actt")
            nc.sync.dma_start(out=actt[:], in_=acts[:, :])
            w1l_q = [load_w1l(0)]
            w1h_q.append(load_w1h(1))
            for o, L in CH[1:]:
                nc.sync.dma_start(out=xth[:, :, :, o:o + L],
                                  in_=xTh[:, :, :, o:o + L])
                nc.sync.dma_start(out=xtl[:, :, :, o:o + L],
                                  in_=xTl[:, :, :, o:o + L])
            if not b1z:
                b1gt = smalls.tile([P, FD], f32, tag="b1gt")
                nc.sync.dma_start(out=b1gt[:], in_=b1g[:, :])
                b1st = smalls.tile([P, FD], f32, tag="b1st")
                nc.sync.dma_start(out=b1st[:], in_=b1s[:, :])
                b1at = smalls.tile([P, FD], f32, tag="b1at")
                nc.sync.dma_start(out=b1at[:], in_=b1a[:, :])
            if not b2z:
                b2t = smalls.tile([P, D_MODEL], f32, tag="b2t")
                nc.sync.dma_start(out=b2t[:], in_=b2bc[:, :])
            wctt = smalls.tile([P, G], f32, tag="wctt")
            nc.sync.dma_start(out=wctt[:], in_=wct[:, :])
            ix12 = smalls.tile([P, 2 * SHARD // 16], i16, tag="ix12")
            nc.sync.dma_start(out=ix12[:], in_=idx12[:, :])

            # mm2 weights, loaded in p2-pairs as background DMAs trickled
            # through the mm1 f-loop.
            bg = []
            w2p_sets = []  # per phase: (hi list, lo list)
            PH = tuple(zip(QO, QWS))
            for ph, (c0, W) in enumerate(PH):
                hi = [None] * FD2
                lo = [None] * FD2
                w2p_sets.append((hi, lo))
                for pp in range(FD2 // 2):
                    def _ldh(pp=pp, ph=ph, c0=c0, W=W, dt=fp8h, W2x=W2h,
                             dst=hi):
                        t = w2p.tile([P, 2, 2, W], dt, tag=f"w2{ph}h_{pp}",
                                     name=f"w2{ph}h_{pp}")
                        nc.sync.dma_start(out=t[:],
                                          in_=W2x[pp][:, :, :, c0:c0 + W])
                        dst[2 * pp] = t
                        dst[2 * pp + 1] = t
                    bg.append(_ldh)
                    def _ldl(pp=pp, ph=ph, c0=c0, W=W, dt=fp8l, W2x=W2l,
                             dst=lo):
                        t = w2p.tile([P, 2, 2, W], dt, tag=f"w2{ph}l_{pp}",
                                     name=f"w2{ph}l_{pp}")
                        nc.sync.dma_start(out=t[:],
                                          in_=W2x[pp][:, :, :, c0:c0 + W])
                        dst[2 * pp] = t
                        dst[2 * pp + 1] = t
                    bg.append(_ldl)

            Erf = mybir.ActivationFunctionType.Erf
            Sig = mybir.ActivationFunctionType.Sigmoid
            mult = mybir.AluOpType.mult
            addop = mybir.AluOpType.add

            def mm1_chunk(fp, o, L, w1ts, tmp):
                """One (f-pair, chunk): 24 DR matmuls into a paired psum
                [P,2,L], then ACT/DVE consumer ops into tmp[:, :, o:o+L]."""
                w1fh, w1fl = w1ts
                ps = psm1.tile([P, 2, 512], mybir.dt.float32, tag="ps1")
                n3 = 3 * KD2
                for j in range(2):
                    i = 0
                    for (wt, xt) in ((w1fh, xth), (w1fh, xtl), (w1fl, xth)):
                        for k2 in range(KD2):
                            nc.tensor.matmul(ps[:, j, :L],
                                             lhsT=wt[:, j, k2, :, :],
                                             rhs=xt[:, k2, :, o:o + L],
                                             start=(i == 0),
                                             stop=(i == n3 - 1),
                                             perf_mode=DR)
                            i += 1
                t1 = actp.tile([P, 2, 512], bf16, tag="t1")
                t2 = actp.tile([P, 2, 512], bf16, tag="t2")
                if b1z:
                    nc.scalar.activation(t1[:, :, :L], ps[:, :, :L], Erf,
                                         bias=0.0, scale=actt[:, 0:1])
                    nc.scalar.activation(t2[:, :, :L], ps[:, :, :L], Sig,
                                         bias=0.0, scale=actt[:, 1:2])
                else:
                    for j in range(2):
                        f = 2 * fp + j
                        nc.scalar.activation(
                            t1[:, j, :L], ps[:, j, :L], Erf,
                            bias=b1gt[:, f:f + 1], scale=actt[:, 0:1])
                        nc.scalar.activation(
                            t2[:, j, :L], ps[:, j, :L], Sig,
                            bias=b1st[:, f:f + 1], scale=actt[:, 1:2])
                # v = 0.5*t1 + t2 ; temp = (ps + 32*b1) * v = 32*aT
                nc.vector.scalar_tensor_tensor(
                    out=t1[:, :, :L], in0=t1[:, :, :L], scalar=0.5,
                    in1=t2[:, :, :L], op0=mult, op1=addop)
                if b1z:
                    nc.vector.scalar_tensor_tensor(
                        out=tmp[:, :, o:o + L], in0=ps[:, :, :L], scalar=0.0,
                        in1=t1[:, :, :L], op0=addop, op1=mult)
                else:
                    for j in range(2):
                        f = 2 * fp + j
                        nc.vector.scalar_tensor_tensor(
                            out=tmp[:, j, o:o + L], in0=ps[:, j, :L],
                            scalar=b1at[:, f:f + 1], in1=t1[:, j, :L],
                            op0=addop, op1=mult)

            def mm2_group(g, w2ts, c0, W, q):
                w2th, w2tl = w2ts
                ps = psm2.tile([P, 512], mybir.dt.float32, tag="ps2")
                n3 = 3 * FD2
                i = 0
                # al (deferred-sub product of the last f-pair) goes LAST
                for (at, wt) in ((ah8, w2th), (ah8, w2tl), (al8, w2th)):
                    for p2 in range(FD2):
                        nc.tensor.matmul(
                            ps[:, :W],
                            lhsT=at[p2][:, :, g * P:(g + 1) * P],
                            rhs=wt[p2][:, p2 % 2, :, c0:c0 + W],
                            start=(i == 0), stop=(i == n3 - 1),
                            perf_mode=DR)
                        i += 1
                snd = sndp.tile([P, 512], bf16, tag="snd")
                nc.vector.tensor_scalar_mul(snd[:, :W], ps[:, :W],
                                            wctt[:, g:g + 1])
                nc.sync.dma_start(out=send_q[q][g * P:(g + 1) * P, :],
                                  in_=snd[:, 0:W])

            # ---- mm1: f-pair outer, chunk inner. e4m3 hi split via gpsimd
            # cast-DMA per pair; e5m2 residual sub deferred one pair so the
            # Pool queue never head-blocks on the cast-DMA's completion sem.
            pending_sub = []
            for fp in range(FD // 2):
                if fp + 1 < FD // 2:
                    w1l_q.append(load_w1l(fp + 1))
                if fp + 2 < FD // 2:
                    w1h_q.append(load_w1h(fp + 2))
                w1ts = (w1h_q[0], w1l_q[0])
                tmp = tmpp.tile([P, 2, SR], bf16, tag="tmp")
                for (o, L) in CH:
                    mm1_chunk(fp, o, L, w1ts, tmp)
                    if bg:
                        bg.pop(0)()
                nc.gpsimd.dma_start(out=ah8[fp][:], in_=tmp[:])  # e4m3 cast
                if pending_sub:
                    pending_sub.pop(0)()
                def _sub(fp=fp, tmp=tmp):
                    nc.gpsimd.tensor_sub(al8[fp][:], tmp[:], ah8[fp][:])
                pending_sub.append(_sub)
                w1h_q.pop(0)
                w1l_q.pop(0)

            while pending_sub:
                pending_sub.pop(0)()
            while bg:
                bg.pop(0)()

            def a2a(q):
                nc.gpsimd.collective_compute(
                    "AllToAll", mybir.AluOpType.bypass,
                    replica_groups=[list(range(NCORES))],
                    ins=[send_q[q][:, :]], outs=[recv_q[q][:, :]])

            y_wrap = y_shard.rearrange("(tg p) d -> p tg d", p=P)

            def combine(q):
                # one gather fetches BOTH contributions of every owned token
                W = QWS[q]
                g1 = cmb.tile([P, 2 * TG, W], bf16, tag=f"g1_{q}",
                              name=f"g1_{q}")
                nc.gpsimd.dma_gather(
                    out_ap=g1[:, :, :], in_ap=recv_q[q][:, :], idxs_ap=ix12[:],
                    num_idxs=2 * SHARD, num_idxs_reg=2 * SHARD, elem_size=W)
                nc.vector.tensor_add(g1[:, 0:TG, :], g1[:, 0:TG, :],
                                     g1[:, TG:2 * TG, :])
                if not b2z:
                    for tg in range(TG):
                        nc.vector.tensor_add(g1[:, tg, :], g1[:, tg, :],
                                             b2t[:, QO[q]:QO[q] + W])
                nc.sync.dma_start(out=y_wrap[:, :, QO[q]:QO[q] + W],
                                  in_=g1[:, 0:TG, :])

            # ---- mm2 in three telescoping phases: each phase's collective
            # and combine overlap the next phase's matmuls; only the last
            # (128-col) chain is exposed at the end.
            for ph, (c0, W) in enumerate(PH):
                for g in range(G):
                    mm2_group(g, w2p_sets[ph], 0, W, ph)
                a2a(ph)
                combine(ph)

    nc.compile()
    return nc


def _route(x_flat, Wg, bg):
    logits = x_flat.astype(np.float32) @ Wg.astype(np.float32) + bg
    order = np.argsort(-logits, axis=1, kind="stable")
    i1, i2 = order[:, 0], order[:, 1]
    s1 = np.take_along_axis(logits, i1[:, None], 1)[:, 0]
    s2 = np.take_along_axis(logits, i2[:, None], 1)[:, 0]
    e = np.exp((s2 - s1).astype(np.float32))
    w1 = 1.0 / (1.0 + e)
    w2 = e * w1
    return i1, i2, w1.astype(np.float32), w2.astype(np.float32)


def _balance_owners(i1, i2):
    """Greedy owner assignment: each token lands in buckets (i1,o),(i2,o)
    of its owner o. Pick o (with fill < SHARD) minimizing the resulting
    max bucket; process tokens routed to heavy experts first. Reaches
    max bucket = ceil(max_e N_e / NCORES) on typical routing."""
    Ne = np.bincount(np.concatenate([i1, i2]), minlength=N_EXPERTS)
    prio = Ne[i1] + Ne[i2]
    idx_order = np.argsort(-prio, kind="stable")
    n = np.zeros((N_EXPERTS, NCORES), np.int64)
    fill = np.zeros(NCORES, np.int64)
    owner = np.full(T, -1, np.int64)
    for t in idx_order:
        a, b = i1[t], i2[t]
        best, bo = None, -1
        for o in range(NCORES):
            if fill[o] >= SHARD:
                continue
            key = (max(n[a, o], n[b, o]) + 1, n[a, o] + n[b, o], fill[o])
            if best is None or key < best:
                best, bo = key, o
        owner[t] = bo
        n[a, bo] += 1
        n[b, bo] += 1
        fill[bo] += 1
    return owner


def _hi_lo(a):
    hi = np.clip(a, -240, 240).astype(_e4)
    lo = (a - hi.astype(np.float32)).astype(_e5)
    return hi, lo


def _prep_weights(W1, W2):
    key = (id(W1), id(W2))
    hit = _wprep_cache.get(key)
    if hit is not None:
        return hit
    W1s = np.asarray(W1, np.float32) * WSCALE
    # [e, fp, p, fi, k2*2*128+...] = W1s[e, (2*k2+j)*128+p, (2*fp+fi)*128+q]
    W1r = (W1s.reshape(N_EXPERTS, KD2, 2, P, FD, P)
           .transpose(0, 4, 3, 1, 2, 5)
           .reshape(N_EXPERTS, FD // 2, 2, P, KD2 * 2 * P)
           .transpose(0, 1, 3, 2, 4))
    W1r = np.ascontiguousarray(W1r)
    W1rh, W1rl = _hi_lo(W1r)
    W2s = np.asarray(W2, np.float32) * WSCALE
    # [e, pp, p, pi, j, d] = W2s[e, (2*(2*pp+pi)+j)*128+p, d]
    W2r = (W2s.reshape(N_EXPERTS, FD2, 2, P, D_MODEL)
           .transpose(0, 1, 3, 2, 4)
           .reshape(N_EXPERTS, FD2 // 2, 2, P, 2, D_MODEL)
           .transpose(0, 1, 3, 2, 4, 5))
    W2r = np.ascontiguousarray(W2r)
    W2rh, W2rl = _hi_lo(W2r)
    _wprep_cache.clear()
    _wprep_cache[key] = (W1rh, W1rl, W2rh, W2rl)
    return _wprep_cache[key]


def _prepare(x, W1, b1, W2, b2, Wg, bg):
    x = np.asarray(x, np.float32)
    b1 = np.asarray(b1, np.float32)
    b2 = np.asarray(b2, np.float32)
    x_flat = np.ascontiguousarray(x.reshape(T, D_MODEL))
    i1, i2, w1, w2 = _route(x_flat, np.asarray(Wg, np.float32),
                            np.asarray(bg, np.float32))
    owner = _balance_owners(i1, i2)
    owned = [np.nonzero(owner == c)[0] for c in range(NCORES)]
    Wq = _prep_weights(W1, W2)

    jobs = {}  # expert -> (ids ascending, wts)
    for e in range(N_EXPERTS):
        sel = (i1 == e) | (i2 == e)
        ids = np.nonzero(sel)[0]
        wts = np.where(i1[ids] == e, w1[ids], w2[ids]).astype(np.float32)
        jobs[e] = (ids, wts)
    return x_flat, jobs, (Wq, b1, b2), owner, owned


def _wrap_idx(r):
    """[n] int -> [128, n/16] int16 (wrapped by 16, replicated 8x)."""
    n = len(r)
    w = np.zeros((16, n // 16), np.int16)
    w[np.arange(n) % 16, np.arange(n) // 16] = r
    return np.tile(w, (8, 1))


def _pass_maps(x_flat, jobs, consts, owner, owned, first_pass=True,
               strict=False):
    (W1rh, W1rl, W2rh, W2rl), b1, b2 = consts
    b1z = not b1.any()
    b2z = not b2.any()

    bucket_count = np.zeros((NCORES, NCORES), np.int64)
    for e in range(NCORES):
        ids, _ = jobs[e]
        own = owner[ids]
        for o in range(NCORES):
            bucket_count[e, o] += (own == o).sum()
    CAP = max(16, int(-(-bucket_count.max() // 16) * 16))
    assert CAP <= MAX_CAP
    SR = NCORES * CAP
    G = SR // P

    # recv row (on the owner) of each token contribution
    src_rows = np.full((T, 2), -1, np.int64)
    slot_of = {}
    for e in range(NCORES):
        ids, _ = jobs[e]
        own = owner[ids]
        ks = np.empty(len(ids), np.int64)
        fill = np.zeros(NCORES, np.int64)
        for o in range(NCORES):
            m = own == o
            nm = int(m.sum())
            ks[m] = fill[o] + np.arange(nm)
            fill[o] += nm
        slot_of[e] = own * CAP + ks
        rows_recv = e * CAP + ks
        which = (src_rows[ids, 0] >= 0).astype(np.int64)
        src_rows[ids, which] = rows_recv
    if strict:
        assert (src_rows >= 0).all()

    sq2 = np.float32(1.0 / np.sqrt(2.0))
    in_maps = []
    for c in range(NCORES):
        e = c
        ids, wts = jobs[e]
        xTc = np.zeros((D_MODEL, SR), np.float32)
        wcol = np.zeros(SR, np.float32)
        if len(ids):
            slots = slot_of[e]
            xTc[:, slots] = x_flat[ids].T
            wcol[slots] = wts / np.float32(WSCALE * WSCALE)
        xh, xl = _hi_lo(xTc)
        # [p, k2, j, n] = x[(2*k2+j)*128+p, n]
        xh = np.ascontiguousarray(
            xh.reshape(KD2, 2, P, SR).transpose(2, 0, 1, 3))
        xl = np.ascontiguousarray(
            xl.reshape(KD2, 2, P, SR).transpose(2, 0, 1, 3))
        even = (e % 2 == 0)
        actsel = np.zeros((P, 2), np.float32)
        actsel[:, 0] = sq2 / WSCALE if even else 0.0
        actsel[:, 1] = 0.0 if even else 1.0 / WSCALE
        my_ids = owned[c]
        r1 = src_rows[my_ids, 0]
        r2 = src_rows[my_ids, 1]
        if not strict:
            # under npass splitting a token's two contributions may land in
            # different passes; point the missing one at a universal pad row
            # (zero on every core).
            pad_slot = _find_pad_row(bucket_count, CAP)
            r1 = np.where(r1 < 0, pad_slot, r1)
            r2 = np.where(r2 < 0, pad_slot, r2)
        im = {
            "xTh": xh, "xTl": xl,
            "W1h": W1rh[e], "W1l": W1rl[e],
            "W2h": W2rh[e], "W2l": W2rl[e],
            "acts": actsel,
            "wct": np.ascontiguousarray(wcol.reshape(G, P).T),
            "idx12": _wrap_idx(np.concatenate([r1, r2])),
        }
        if not b1z:
            b1_cols = np.ascontiguousarray(b1[e].reshape(FD, P).T)  # [P, FD]
            b1gv = b1_cols * sq2 if even else np.zeros((P, FD), np.float32)
            b1sv = np.zeros((P, FD), np.float32) if even else b1_cols
            im["b1g"] = np.ascontiguousarray(b1gv)
            im["b1s"] = np.ascontiguousarray(b1sv)
            im["b1a"] = np.ascontiguousarray(b1_cols * np.float32(WSCALE))
        if not b2z:
            b2v = b2[e] if first_pass else np.zeros(D_MODEL, np.float32)
            im["b2bc"] = np.ascontiguousarray(
                np.broadcast_to(b2v, (P, D_MODEL)).astype(np.float32))
        in_maps.append(im)
    return (CAP, b1z, b2z), in_maps


def _find_pad_row(bucket_count, CAP):
    """Recv row index that is a zero pad slot on every core: pick (e, k)
    with k >= max_o bucket_count[e, o]."""
    per_e_max = bucket_count.max(axis=1)
    e = int(per_e_max.argmin())
    k = int(per_e_max[e])
    assert k < CAP, "no universal pad slot (all buckets full)"
    return e * CAP + k


def make_in_maps(x, W1, b1, W2, b2, Wg, bg):
    x_flat, jobs, consts, owner, owned = _prepare(x, W1, b1, W2, b2, Wg, bg)
    return _pass_maps(x_flat, jobs, consts, owner, owned, strict=True)


def get_program(key):
    if key not in _prog_cache:
        _prog_cache[key] = _build_program(*key)
    return _prog_cache[key]


def kernel(x, W1, b1, W2, b2, Wg, bg):
    from concourse.bass_utils import run_bass_kernel_spmd

    x_flat, jobs, consts, owner, owned = _prepare(x, W1, b1, W2, b2, Wg, bg)
    maxbucket = 0
    for e in range(N_EXPERTS):
        own = owner[jobs[e][0]]
        if len(own):
            maxbucket = max(maxbucket, int(np.bincount(own).max()))
    npass = max(1, -(-maxbucket // MAX_CAP))
    out = None
    for p in range(npass):
        jobs_p = {e: (ids[p::npass], wts[p::npass])
                  for e, (ids, wts) in jobs.items()}
        key, in_maps = _pass_maps(x_flat, jobs_p, consts, owner, owned,
                                  first_pass=(p == 0), strict=(npass == 1))
        nc = get_program(key)
        res = run_bass_kernel_spmd(nc, in_maps, list(range(NCORES)))
        full = np.empty((T, D_MODEL), np.float32)
        for c in range(NCORES):
            full[owned[c]] = res.results[c]["y_shard"].astype(np.float32)
        out = full if out is None else out + full
    return np.ascontiguousarray(out.reshape(B, S, D_MODEL))


# revision 41
# speedup vs baseline: 1.2089x; 1.2089x over previous
"""MoE routing kernel for Trainium2, 8 NeuronCores, expert-parallel.

V3: V2 (3-stream error-compensated fp8 DoubleRow matmuls) plus:

- Owner rebalancing: token "ownership" (which core outputs which token's
  combined y) is a free permutation. A greedy balancer assigns owners so
  every (expert, owner) bucket count approaches ceil(N_e/8), shrinking the
  bucket pad CAP from max_bucket (160 here) to ~144 => SR=8*CAP drops ~10%
  and ALL PE work (both matmuls scale with SR) drops with it. The host
  un-permutes rows on output assembly.
- mm1 consumer rebalance (b1==0 fast path): the two ACT ops and the two
  DVE stt ops run PAIRED over [P, 2, L] (per f-pair, halving per-op init
  overhead); the e4m3 hi split is a gpsimd cast-DMA (SWDGE dtype-cast,
  ~1us Pool-engine descgen instead of ~3.2us of Pool ALU copy); only the
  e5m2 residual sub stays on Pool ALU, software-pipelined one f-pair
  behind the cast so the Pool queue never head-blocks on the DMA sem.
  Per-f-pair engine busy (SR=1152): PE 5760, ACT ~4950, DVE ~3970,
  Pool ~4300 -> PE-bound.
- mm2 stream order (ah@W2h, ah@W2l, al@W2h) so the last-produced operand
  (al of the final f-pair, which trails its deferred sub) is needed last.
- b2==0 fast path drops the 4 per-tg bias adds from every combine tail.
- First x chunk is 128 columns so the first matmul starts ~2us earlier.

Matmul scheme (unchanged from V2): A = Ah + Al with Ah=e4m3(A),
Al=e5m2(A-Ah); A@B ~= Ah@Bh + Al@Bh + Ah@Bl as fp8 DoubleRow (0.5 cyc/col,
256-row contraction) = 1.5 cyc/col per 256 rows vs bf16's 2.0. W1/W2
pre-scaled by 32 for e4m3 normal range; activations computed as
temp = 32*act(h) via Erf/Sigmoid table tricks (function choice is per-core
DATA through scale operands); psum2 descaled by wcol/1024 into the bf16
all-to-all payload; owners gather their two contribution rows per token.
"""

import numpy as np
import ml_dtypes

D_MODEL, D_FF, N_EXPERTS, TOP_K = 1024, 4096, 8, 2
B, S = 2, 2048
T = B * S
NCORES = 8
P = 128
SHARD = T // NCORES     # 512 tokens owned per core
FD = D_FF // P          # 32 f-tiles
FD2 = FD // 2           # 16 f-pairs
KD = D_MODEL // P       # 8 k-tiles (d_model)
KD2 = KD // 2           # 4 k-pairs
TG = SHARD // P         # 4 owned-token tiles
MAX_CAP = 160           # SR = 8*CAP <= 1280 (SBUF residency bound)
WSCALE = 32.0           # fp8 pre-scale for W1/W2

_prog_cache = {}
_wprep_cache = {}

_bf16 = ml_dtypes.bfloat16
_e4 = ml_dtypes.float8_e4m3
_e5 = ml_dtypes.float8_e5m2


def _chunks(n):
    """Even chunks <= 384 so the psum->ACT->DVE consumer chain of one chunk
    fits under two chunks of PE run-ahead (psm1 bufs=3)."""
    k = -(-n // 384)
    per = -(-n // (16 * k)) * 16
    sizes = [per] * (k - 1) + [n - per * (k - 1)]
    out, o = [], 0
    for L in sizes:
        out.append((o, L))
        o += L
    return out


def _chunks_first(n):
    """fp0 variant: tiny leading chunk so the first matmul starts as soon
    as a 128-column x slice has landed."""
    ch = _chunks(n)
    (o0, L0) = ch[0]
    if L0 > 256:
        return [(0, 128), (128, L0 - 128)] + ch[1:]
    return ch


def _build_program(CAP, b1z, b2z):
    import concourse.tile as tile
    from concourse import bacc, mybir, library_config

    f32 = mybir.dt.float32
    bf16 = mybir.dt.bfloat16
    fp8h = mybir.dt.float8e4
    fp8l = mybir.dt.float8e5
    i16 = mybir.dt.int16
    DR = mybir.MatmulPerfMode.DoubleRow
    SR = NCORES * CAP
    G = SR // P
    # blocks double as processing chunks AND block-major x DMA segments
    CH = _chunks_first(SR)
    CW = max(L for _, L in CH)

    nc = bacc.Bacc("TRN2", target_bir_lowering=False, debug=False,
                   num_devices=NCORES)

    # block-major: per partition, each block b=(o,L) is a contiguous
    # [KD2, 2, L] segment -> x DMAs move 1-3 KB contiguous rows (half the
    # descriptor cost of column-sliced loads)
    xTh = nc.dram_tensor("xTh", [P, KD2 * 2 * SR], fp8h,
                         kind="ExternalInput").ap()
    xTl = nc.dram_tensor("xTl", [P, KD2 * 2 * SR], fp8l,
                         kind="ExternalInput").ap()
    W1h = nc.dram_tensor("W1h", [FD // 2, P, 2, KD2 * 2 * P], fp8h,
                         kind="ExternalInput").ap()
    W1l = nc.dram_tensor("W1l", [FD // 2, P, 2, KD2 * 2 * P], fp8l,
                         kind="ExternalInput").ap()
    W2h = nc.dram_tensor("W2h", [FD2 // 2, P, 2, 2, D_MODEL], fp8h,
                         kind="ExternalInput").ap()
    W2l = nc.dram_tensor("W2l", [FD2 // 2, P, 2, 2, D_MODEL], fp8l,
                         kind="ExternalInput").ap()
    acts = nc.dram_tensor("acts", [P, 2], f32, kind="ExternalInput").ap()
    if not b1z:
        b1g = nc.dram_tensor("b1g", [P, FD], f32, kind="ExternalInput").ap()
        b1s = nc.dram_tensor("b1s", [P, FD], f32, kind="ExternalInput").ap()
        b1a = nc.dram_tensor("b1a", [P, FD], f32, kind="ExternalInput").ap()
    if not b2z:
        b2bc = nc.dram_tensor("b2bc", [P, D_MODEL], f32,
                              kind="ExternalInput").ap()
    wct = nc.dram_tensor("wct", [P, G], f32, kind="ExternalInput").ap()
    idx12 = nc.dram_tensor("idx12", [P, 2 * SHARD // 16], i16,
                           kind="ExternalInput").ap()
    y_shard = nc.dram_tensor("y_shard", [SHARD, D_MODEL], bf16,
                             kind="ExternalOutput").ap()

    # collective payload split: telescoping phases; each phase's
    # send/a2a/gather/combine chain hides under the next (wider-window)
    # phase's matmuls, leaving only the final 128-col chain exposed
    QWS = (512, 256, 128, 128)
    QO = (0, 512, 768, 896)
    NQ = len(QWS)
    send_q = [nc.dram_tensor(f"send_q{q}", [SR, QWS[q]], bf16).ap()
              for q in range(NQ)]
    recv_q = [nc.dram_tensor(f"recv_q{q}", [SR, QWS[q]], bf16).ap()
              for q in range(NQ)]

    with tile.TileContext(nc) as tc:
        with (
            tc.tile_pool(name="xtp", bufs=1) as xtp,
            tc.tile_pool(name="atp", bufs=1) as atp,
            tc.tile_pool(name="w1ph", bufs=3) as w1ph,
            tc.tile_pool(name="w1p", bufs=2) as w1p,
            tc.tile_pool(name="w2p", bufs=1) as w2p,
            tc.tile_pool(name="smalls", bufs=1) as smalls,
            tc.tile_pool(name="actp", bufs=3) as actp,
            tc.tile_pool(name="sndp", bufs=3) as sndp,
            tc.tile_pool(name="tmpp", bufs=2) as tmpp,
            tc.tile_pool(name="cmb", bufs=1) as cmb,
        ):
            # PSUM pools open sequentially: mm1 gets all 8 banks (4 paired
            # buffers), released before mm2's pool opens.
            psm = {}
            nc.gpsimd.load_library(library_config.mlp)

            xth = []
            xtl = []
            for b, (o, L) in enumerate(CH):
                xth.append(xtp.tile([P, KD2, 2, L], fp8h, tag=f"xh{b}",
                                    name=f"xh{b}"))
                xtl.append(xtp.tile([P, KD2, 2, L], fp8l, tag=f"xl{b}",
                                    name=f"xl{b}"))

            def load_x(b):
                (o, L) = CH[b]
                off = KD2 * 2 * o
                n = KD2 * 2 * L
                nc.sync.dma_start(out=xth[b][:], in_=xTh[:, off:off + n]
                                  .rearrange("p (a c q) -> p a c q",
                                             a=KD2, c=2))
                nc.sync.dma_start(out=xtl[b][:], in_=xTl[:, off:off + n]
                                  .rearrange("p (a c q) -> p a c q",
                                             a=KD2, c=2))

            ah8, al8 = [], []
            for p2 in range(FD2):
                ah8.append(atp.tile([P, 2, SR], fp8h, tag=f"ah{p2}",
                                    name=f"ah{p2}"))
                al8.append(atp.tile([P, 2, SR], fp8l, tag=f"al{p2}",
                                    name=f"al{p2}"))

            def load_w1h(fp):
                w1fh = w1ph.tile([P, 2, KD2, 2, P], fp8h, tag="w1fh")
                nc.sync.dma_start(out=w1fh[:], in_=W1h[fp].rearrange(
                    "p c (a b q) -> p c a b q", a=KD2, b=2))
                return w1fh

            def load_w1l(fp):
                w1fl = w1p.tile([P, 2, KD2, 2, P], fp8l, tag="w1fl")
                nc.sync.dma_start(out=w1fl[:], in_=W1l[fp].rearrange(
                    "p c (a b q) -> p c a b q", a=KD2, b=2))
                return w1fl

            # FIFO prefetch: hi tiles 2 pairs ahead, lo 1 pair ahead.
            # DMA issue order front-loads what the PE needs first: W1h(0),
            # x chunk0 (128 cols), act scales, then the rest.
            w1h_q = [load_w1h(0)]
            load_x(0)
            load_x(1)
            actt = smalls.tile([P, 2], f32, tag="actt")
            nc.sync.dma_start(out=actt[:], in_=acts[:, :])
            for b in range(2, len(CH)):
                load_x(b)
            w1l_q = [load_w1l(0)]
            w1h_q.append(load_w1h(1))
            if not b1z:
                b1gt = smalls.tile([P, FD], f32, tag="b1gt")
                nc.sync.dma_start(out=b1gt[:], in_=b1g[:, :])
                b1st = smalls.tile([P, FD], f32, tag="b1st")
                nc.sync.dma_start(out=b1st[:], in_=b1s[:, :])
                b1at = smalls.tile([P, FD], f32, tag="b1at")
                nc.sync.dma_start(out=b1at[:], in_=b1a[:, :])
            if not b2z:
                b2t = smalls.tile([P, D_MODEL], f32, tag="b2t")
                nc.sync.dma_start(out=b2t[:], in_=b2bc[:, :])
            wctt = smalls.tile([P, G], f32, tag="wctt")
            nc.sync.dma_start(out=wctt[:], in_=wct[:, :])
            ix12 = smalls.tile([P, 2 * SHARD // 16], i16, tag="ix12")
            nc.sync.dma_start(out=ix12[:], in_=idx12[:, :])

            # mm2 weights, loaded in p2-pairs as background DMAs trickled
            # through the mm1 f-loop.
            bg = []
            w2p_sets = []  # per phase: (hi list, lo list)
            PH = tuple(zip(QO, QWS))
            for ph, (c0, W) in enumerate(PH):
                hi = [None] * FD2
                lo = [None] * FD2
                w2p_sets.append((hi, lo))
                for pp in range(FD2 // 2):
                    def _ldh(pp=pp, ph=ph, c0=c0, W=W, dt=fp8h, W2x=W2h,
                             dst=hi):
                        t = w2p.tile([P, 2, 2, W], dt, tag=f"w2{ph}h_{pp}",
                                     name=f"w2{ph}h_{pp}")
                        nc.sync.dma_start(out=t[:],
                                          in_=W2x[pp][:, :, :, c0:c0 + W])
                        dst[2 * pp] = t
                        dst[2 * pp + 1] = t
                    bg.append(_ldh)
                    def _ldl(pp=pp, ph=ph, c0=c0, W=W, dt=fp8l, W2x=W2l,
                             dst=lo):
                        t = w2p.tile([P, 2, 2, W], dt, tag=f"w2{ph}l_{pp}",
                                     name=f"w2{ph}l_{pp}")
                        nc.sync.dma_start(out=t[:],
                                          in_=W2x[pp][:, :, :, c0:c0 + W])
                        dst[2 * pp] = t
                        dst[2 * pp + 1] = t
                    bg.append(_ldl)

            Erf = mybir.ActivationFunctionType.Erf
            Sig = mybir.ActivationFunctionType.Sigmoid
            Tnh = mybir.ActivationFunctionType.Tanh
            mult = mybir.AluOpType.mult
            addop = mybir.AluOpType.add

            def mm1_chunk(fp, b, w1ts, tmp):
                """One (f-pair, block): 24 DR matmuls into a paired psum
                [P,2,L], then ACT/DVE consumer ops into tmp[:, :, o:o+L]."""
                (o, L) = CH[b]
                w1fh, w1fl = w1ts
                # 512-wide pair regardless of chunk: each j-half must sit
                # exactly on a 2KB PSUM bank (accumulation cannot straddle)
                ps = psm["p1"].tile([P, 2, 512], mybir.dt.float32, tag="ps1")
                n3 = 3 * KD2
                for j in range(2):
                    i = 0
                    for (wt, xt) in ((w1fh, xth[b]), (w1fh, xtl[b]),
                                     (w1fl, xth[b])):
                        for k2 in range(KD2):
                            nc.tensor.matmul(ps[:, j, :L],
                                             lhsT=wt[:, j, k2, :, :],
                                             rhs=xt[:, k2, :, :],
                                             start=(i == 0),
                                             stop=(i == n3 - 1),
                                             perf_mode=DR)
                            i += 1
                t1 = actp.tile([P, 2, CW], bf16, tag="t1")
                t2 = actp.tile([P, 2, CW], bf16, tag="t2")
                if b1z:
                    # Both parities are v = 0.5 + 0.5*f(.): even f=erf
                    # (Phi = 0.5+0.5*erf(h/sqrt2)), odd f=tanh
                    # (sigma = 0.5+0.5*tanh(h/2)); the inactive parity's
                    # scale is 0 so its term is f(0)=0. Then
                    #   u = t1 + t2            (TensorTensor add: 2x mode)
                    #   temp2 = (u + 1) * ps = 2 * 32 * act  (descaled in wct)
                    nc.scalar.activation(t1[:, :, :L], ps[:, :, :L], Erf,
                                         bias=0.0, scale=actt[:, 0:1])
                    nc.scalar.activation(t2[:, :, :L], ps[:, :, :L], Tnh,
                                         bias=0.0, scale=actt[:, 1:2])
                    nc.vector.tensor_add(t1[:, :, :L], t1[:, :, :L],
                                         t2[:, :, :L])
                    nc.vector.scalar_tensor_tensor(
                        out=tmp[:, :, o:o + L], in0=t1[:, :, :L], scalar=1.0,
                        in1=ps[:, :, :L], op0=addop, op1=mult)
                else:
                    for j in range(2):
                        f = 2 * fp + j
                        nc.scalar.activation(
                            t1[:, j, :L], ps[:, j, :L], Erf,
                            bias=b1gt[:, f:f + 1], scale=actt[:, 0:1])
                        nc.scalar.activation(
                            t2[:, j, :L], ps[:, j, :L], Sig,
                            bias=b1st[:, f:f + 1], scale=actt[:, 1:2])
                    # v = 0.5*t1 + t2 ; temp = (ps + 32*b1) * v = 32*aT
                    nc.vector.scalar_tensor_tensor(
                        out=t1[:, :, :L], in0=t1[:, :, :L], scalar=0.5,
                        in1=t2[:, :, :L], op0=mult, op1=addop)
                    for j in range(2):
                        f = 2 * fp + j
                        nc.vector.scalar_tensor_tensor(
                            out=tmp[:, j, o:o + L], in0=ps[:, j, :L],
                            scalar=b1at[:, f:f + 1], in1=t1[:, j, :L],
                            op0=addop, op1=mult)

            def mm2_group(g, w2ts, c0, W, q):
                w2th, w2tl = w2ts
                ps = psm["p2"].tile([P, 512], mybir.dt.float32, tag="ps2")
                n3 = 3 * FD2
                i = 0
                # al (deferred-sub product of the last f-pair) goes LAST
                for (at, wt) in ((ah8, w2th), (ah8, w2tl), (al8, w2th)):
                    for p2 in range(FD2):
                        nc.tensor.matmul(
                            ps[:, :W],
                            lhsT=at[p2][:, :, g * P:(g + 1) * P],
                            rhs=wt[p2][:, p2 % 2, :, c0:c0 + W],
                            start=(i == 0), stop=(i == n3 - 1),
                            perf_mode=DR)
                        i += 1
                snd = sndp.tile([P, 512], bf16, tag="snd")
                nc.vector.tensor_scalar_mul(snd[:, :W], ps[:, :W],
                                            wctt[:, g:g + 1])
                nc.sync.dma_start(out=send_q[q][g * P:(g + 1) * P, :],
                                  in_=snd[:, 0:W])

            # ---- mm1: f-pair outer, chunk inner. e4m3 hi split via gpsimd
            # cast-DMA per pair; e5m2 residual sub issued at the START of the
            # NEXT pair (its dep -- the cast transfer -- clears mid-pair, so
            # the Pool engine never head-blocks and tmp frees a pair early).
            # The LAST pair splits per-chunk on DVE+Pool ALU instead, so
            # al8[last] lands ~1.4us after the final stt2 and mm2's trailing
            # al stream never waits.
            pending_sub = []
            last_fp = FD // 2 - 1
            psm1_cm = tc.tile_pool(name="psm1", bufs=4, space="PSUM")
            psm["p1"] = psm1_cm.__enter__()
            for fp in range(FD // 2):
                if fp + 1 < FD // 2:
                    w1l_q.append(load_w1l(fp + 1))
                if fp + 2 < FD // 2:
                    w1h_q.append(load_w1h(fp + 2))
                if pending_sub:
                    pending_sub.pop(0)()
                w1ts = (w1h_q[0], w1l_q[0])
                tmp = tmpp.tile([P, 2, SR], bf16, tag="tmp")
                # last pair: biggest chunk first, smallest last, so the final
                # consumer chain (gating psm1's release to psm2) is short
                border = (range(len(CH) - 1, -1, -1) if fp == last_fp
                          else range(len(CH)))
                for b in border:
                    mm1_chunk(fp, b, w1ts, tmp)
                    if fp >= 1 and bg:
                        bg.pop(0)()
                nc.gpsimd.dma_start(out=ah8[fp][:], in_=tmp[:])  # e4m3 cast
                # residual sub split ~1/4 DVE : 3/4 Pool (GPSIMD "Add" runs
                # at 0.42 efficiency; this evens both engines at ~80% of the
                # PE's per-pair rate)
                Ld = (SR // 4) & ~15
                def _sub(fp=fp, tmp=tmp, Ld=Ld):
                    nc.vector.tensor_sub(al8[fp][:, :, 0:Ld],
                                         tmp[:, :, 0:Ld],
                                         ah8[fp][:, :, 0:Ld])
                    nc.gpsimd.tensor_sub(al8[fp][:, :, Ld:],
                                         tmp[:, :, Ld:],
                                         ah8[fp][:, :, Ld:])
                pending_sub.append(_sub)
                w1h_q.pop(0)
                w1l_q.pop(0)

            # release mm1's psum pool BEFORE the trailing subs: the release
            # drains only the ACT/DVE/PE readers (~2us after the last
            # matmul), not the final residual subs, so mm2 starts while
            # sub(last) still runs (its al feeds mm2's trailing stream).
            psm1_cm.__exit__(None, None, None)
            while pending_sub:
                pending_sub.pop(0)()
            while bg:
                bg.pop(0)()
            psm2_cm = tc.tile_pool(name="psm2", bufs=2, space="PSUM")
            psm["p2"] = psm2_cm.__enter__()

            def a2a(q):
                nc.gpsimd.collective_compute(
                    "AllToAll", mybir.AluOpType.bypass,
                    replica_groups=[list(range(NCORES))],
                    ins=[send_q[q][:, :]], outs=[recv_q[q][:, :]])

            y_wrap = y_shard.rearrange("(tg p) d -> p tg d", p=P)

            def combine(q):
                # one gather fetches BOTH contributions of every owned token
                W = QWS[q]
                g1 = cmb.tile([P, 2 * TG, W], bf16, tag=f"g1_{q}",
                              name=f"g1_{q}")
                nc.gpsimd.dma_gather(
                    out_ap=g1[:, :, :], in_ap=recv_q[q][:, :], idxs_ap=ix12[:],
                    num_idxs=2 * SHARD, num_idxs_reg=2 * SHARD, elem_size=W)
                nc.vector.tensor_add(g1[:, 0:TG, :], g1[:, 0:TG, :],
                                     g1[:, TG:2 * TG, :])
                if not b2z:
                    for tg in range(TG):
                        nc.vector.tensor_add(g1[:, tg, :], g1[:, tg, :],
                                             b2t[:, QO[q]:QO[q] + W])
                nc.sync.dma_start(out=y_wrap[:, :, QO[q]:QO[q] + W],
                                  in_=g1[:, 0:TG, :])

            # ---- mm2 in telescoping phases: each phase's collective and
            # combine overlap the next phase's matmuls; only the last
            # (128-col) chain is exposed at the end.
            for ph, (c0, W) in enumerate(PH):
                for g in range(G):
                    mm2_group(g, w2p_sets[ph], 0, W, ph)
                a2a(ph)
                combine(ph)
            psm2_cm.__exit__(None, None, None)

    nc.compile()
    return nc


def _route(x_flat, Wg, bg):
    logits = x_flat.astype(np.float32) @ Wg.astype(np.float32) + bg
    order = np.argsort(-logits, axis=1, kind="stable")
    i1, i2 = order[:, 0], order[:, 1]
    s1 = np.take_along_axis(logits, i1[:, None], 1)[:, 0]
    s2 = np.take_along_axis(logits, i2[:, None], 1)[:, 0]
    e = np.exp((s2 - s1).astype(np.float32))
    w1 = 1.0 / (1.0 + e)
    w2 = e * w1
    return i1, i2, w1.astype(np.float32), w2.astype(np.float32)


def _balance_owners(i1, i2):
    """Greedy owner assignment: each token lands in buckets (i1,o),(i2,o)
    of its owner o. Pick o (with fill < SHARD) minimizing the resulting
    max bucket; process tokens routed to heavy experts first. Reaches
    max bucket = ceil(max_e N_e / NCORES) on typical routing."""
    Ne = np.bincount(np.concatenate([i1, i2]), minlength=N_EXPERTS)
    prio = Ne[i1] + Ne[i2]
    idx_order = np.argsort(-prio, kind="stable")
    n = np.zeros((N_EXPERTS, NCORES), np.int64)
    fill = np.zeros(NCORES, np.int64)
    owner = np.full(T, -1, np.int64)
    for t in idx_order:
        a, b = i1[t], i2[t]
        best, bo = None, -1
        for o in range(NCORES):
            if fill[o] >= SHARD:
                continue
            key = (max(n[a, o], n[b, o]) + 1, n[a, o] + n[b, o], fill[o])
            if best is None or key < best:
                best, bo = key, o
        owner[t] = bo
        n[a, bo] += 1
        n[b, bo] += 1
        fill[bo] += 1
    return owner


def _hi_lo(a):
    hi = np.clip(a, -240, 240).astype(_e4)
    lo = (a - hi.astype(np.float32)).astype(_e5)
    return hi, lo


def _prep_weights(W1, W2):
    key = (id(W1), id(W2))
    hit = _wprep_cache.get(key)
    if hit is not None:
        return hit
    W1s = np.asarray(W1, np.float32) * WSCALE
    # [e, fp, p, fi, k2*2*128+...] = W1s[e, (2*k2+j)*128+p, (2*fp+fi)*128+q]
    W1r = (W1s.reshape(N_EXPERTS, KD2, 2, P, FD, P)
           .transpose(0, 4, 3, 1, 2, 5)
           .reshape(N_EXPERTS, FD // 2, 2, P, KD2 * 2 * P)
           .transpose(0, 1, 3, 2, 4))
    W1r = np.ascontiguousarray(W1r)
    W1rh, W1rl = _hi_lo(W1r)
    W2s = np.asarray(W2, np.float32) * WSCALE
    # [e, pp, p, pi, j, d] = W2s[e, (2*(2*pp+pi)+j)*128+p, d]
    W2r = (W2s.reshape(N_EXPERTS, FD2, 2, P, D_MODEL)
           .transpose(0, 1, 3, 2, 4)
           .reshape(N_EXPERTS, FD2 // 2, 2, P, 2, D_MODEL)
           .transpose(0, 1, 3, 2, 4, 5))
    W2r = np.ascontiguousarray(W2r)
    W2rh, W2rl = _hi_lo(W2r)
    _wprep_cache.clear()
    _wprep_cache[key] = (W1rh, W1rl, W2rh, W2rl)
    return _wprep_cache[key]


def _prepare(x, W1, b1, W2, b2, Wg, bg):
    x = np.asarray(x, np.float32)
    b1 = np.asarray(b1, np.float32)
    b2 = np.asarray(b2, np.float32)
    x_flat = np.ascontiguousarray(x.reshape(T, D_MODEL))
    i1, i2, w1, w2 = _route(x_flat, np.asarray(Wg, np.float32),
                            np.asarray(bg, np.float32))
    owner = _balance_owners(i1, i2)
    owned = [np.nonzero(owner == c)[0] for c in range(NCORES)]
    Wq = _prep_weights(W1, W2)

    jobs = {}  # expert -> (ids ascending, wts)
    for e in range(N_EXPERTS):
        sel = (i1 == e) | (i2 == e)
        ids = np.nonzero(sel)[0]
        wts = np.where(i1[ids] == e, w1[ids], w2[ids]).astype(np.float32)
        jobs[e] = (ids, wts)
    return x_flat, jobs, (Wq, b1, b2), owner, owned


def _wrap_idx(r):
    """[n] int -> [128, n/16] int16 (wrapped by 16, replicated 8x)."""
    n = len(r)
    w = np.zeros((16, n // 16), np.int16)
    w[np.arange(n) % 16, np.arange(n) // 16] = r
    return np.tile(w, (8, 1))


def _pass_maps(x_flat, jobs, consts, owner, owned, first_pass=True,
               strict=False):
    (W1rh, W1rl, W2rh, W2rl), b1, b2 = consts
    b1z = not b1.any()
    b2z = not b2.any()

    # b1z path emits temp2 = 2*32*act (the tanh/erf +1 trick), so the psum2
    # descale absorbs an extra factor of 2.
    wdiv = np.float32(WSCALE * WSCALE * (2.0 if b1z else 1.0))

    bucket_count = np.zeros((NCORES, NCORES), np.int64)
    for e in range(NCORES):
        ids, _ = jobs[e]
        own = owner[ids]
        for o in range(NCORES):
            bucket_count[e, o] += (own == o).sum()
    CAP = max(16, int(-(-bucket_count.max() // 16) * 16))
    assert CAP <= MAX_CAP
    SR = NCORES * CAP
    G = SR // P

    # recv row (on the owner) of each token contribution
    src_rows = np.full((T, 2), -1, np.int64)
    slot_of = {}
    for e in range(NCORES):
        ids, _ = jobs[e]
        own = owner[ids]
        ks = np.empty(len(ids), np.int64)
        fill = np.zeros(NCORES, np.int64)
        for o in range(NCORES):
            m = own == o
            nm = int(m.sum())
            ks[m] = fill[o] + np.arange(nm)
            fill[o] += nm
        slot_of[e] = own * CAP + ks
        rows_recv = e * CAP + ks
        which = (src_rows[ids, 0] >= 0).astype(np.int64)
        src_rows[ids, which] = rows_recv
    if strict:
        assert (src_rows >= 0).all()

    sq2 = np.float32(1.0 / np.sqrt(2.0))
    in_maps = []
    for c in range(NCORES):
        e = c
        ids, wts = jobs[e]
        xTc = np.zeros((D_MODEL, SR), np.float32)
        wcol = np.zeros(SR, np.float32)
        if len(ids):
            slots = slot_of[e]
            xTc[:, slots] = x_flat[ids].T
            wcol[slots] = wts / wdiv
        xh, xl = _hi_lo(xTc)
        # [p, k2, j, n] = x[(2*k2+j)*128+p, n], then block-major flattened
        # to match the kernel's per-block x tiles
        xh = xh.reshape(KD2, 2, P, SR).transpose(2, 0, 1, 3)
        xl = xl.reshape(KD2, 2, P, SR).transpose(2, 0, 1, 3)
        blocks = _chunks_first(SR)
        xh = np.concatenate(
            [xh[:, :, :, o:o + L].reshape(P, -1) for (o, L) in blocks], 1)
        xl = np.concatenate(
            [xl[:, :, :, o:o + L].reshape(P, -1) for (o, L) in blocks], 1)
        xh = np.ascontiguousarray(xh)
        xl = np.ascontiguousarray(xl)
        even = (e % 2 == 0)
        actsel = np.zeros((P, 2), np.float32)
        actsel[:, 0] = sq2 / WSCALE if even else 0.0
        if b1z:
            # odd experts: sigma(h) = 0.5 + 0.5*tanh(h/2), h = ps/32
            actsel[:, 1] = 0.0 if even else 0.5 / WSCALE
        else:
            actsel[:, 1] = 0.0 if even else 1.0 / WSCALE
        my_ids = owned[c]
        r1 = src_rows[my_ids, 0]
        r2 = src_rows[my_ids, 1]
        if not strict:
            # under npass splitting a token's two contributions may land in
            # different passes; point the missing one at a universal pad row
            # (zero on every core).
            pad_slot = _find_pad_row(bucket_count, CAP)
            r1 = np.where(r1 < 0, pad_slot, r1)
            r2 = np.where(r2 < 0, pad_slot, r2)
        im = {
            "xTh": xh, "xTl": xl,
            "W1h": W1rh[e], "W1l": W1rl[e],
            "W2h": W2rh[e], "W2l": W2rl[e],
            "acts": actsel,
            "wct": np.ascontiguousarray(wcol.reshape(G, P).T),
            "idx12": _wrap_idx(np.concatenate([r1, r2])),
        }
        if not b1z:
            b1_cols = np.ascontiguousarray(b1[e].reshape(FD, P).T)  # [P, FD]
            b1gv = b1_cols * sq2 if even else np.zeros((P, FD), np.float32)
            b1sv = np.zeros((P, FD), np.float32) if even else b1_cols
            im["b1g"] = np.ascontiguousarray(b1gv)
            im["b1s"] = np.ascontiguousarray(b1sv)
            im["b1a"] = np.ascontiguousarray(b1_cols * np.float32(WSCALE))
        if not b2z:
            b2v = b2[e] if first_pass else np.zeros(D_MODEL, np.float32)
            im["b2bc"] = np.ascontiguousarray(
                np.broadcast_to(b2v, (P, D_MODEL)).astype(np.float32))
        in_maps.append(im)
    return (CAP, b1z, b2z), in_maps


def _find_pad_row(bucket_count, CAP):
    """Recv row index that is a zero pad slot on every core: pick (e, k)
    with k >= max_o bucket_count[e, o]."""
    per_e_max = bucket_count.max(axis=1)
    e = int(per_e_max.argmin())
    k = int(per_e_max[e])
    assert k < CAP, "no universal pad slot (all buckets full)"
    return e * CAP + k


def make_in_maps(x, W1, b1, W2, b2, Wg, bg):
    x_flat, jobs, consts, owner, owned = _prepare(x, W1, b1, W2, b2, Wg, bg)
    return _pass_maps(x_flat, jobs, consts, owner, owned, strict=True)


def get_program(key):
    if key not in _prog_cache:
        _prog_cache[key] = _build_program(*key)
    return _prog_cache[key]


def kernel(x, W1, b1, W2, b2, Wg, bg):
    from concourse.bass_utils import run_bass_kernel_spmd

    x_flat, jobs, consts, owner, owned = _prepare(x, W1, b1, W2, b2, Wg, bg)
    maxbucket = 0
    for e in range(N_EXPERTS):
        own = owner[jobs[e][0]]
        if len(own):
            maxbucket = max(maxbucket, int(np.bincount(own).max()))
    npass = max(1, -(-maxbucket // MAX_CAP))
    out = None
    for p in range(npass):
        jobs_p = {e: (ids[p::npass], wts[p::npass])
                  for e, (ids, wts) in jobs.items()}
        key, in_maps = _pass_maps(x_flat, jobs_p, consts, owner, owned,
                                  first_pass=(p == 0), strict=(npass == 1))
        nc = get_program(key)
        res = run_bass_kernel_spmd(nc, in_maps, list(range(NCORES)))
        full = np.empty((T, D_MODEL), np.float32)
        for c in range(NCORES):
            full[owned[c]] = res.results[c]["y_shard"].astype(np.float32)
        out = full if out is None else out + full
    return np.ascontiguousarray(out.reshape(B, S, D_MODEL))
